# revision 30
# baseline (speedup 1.0000x reference)
"""DenseGAT layer (top-16 sparsified, 4 heads) as a Bass/Tile kernel on 8
Trainium2 NeuronCores.

v3 design — sync-free (no collective, no shared DRAM, no cross-core traffic):

Host-side prep (weight packing only, no data-dependent compute):
  - xT   = x.T as bf16 [1024, 4096] (full, replicated to every core)
  - waug = [W.T | w_src | w_dst] as bf16 [1024, 520], where
    w_src_h = W_h.T @ a_src, w_dst_h = W_h.T @ a_dst (pre-contracted `a`),
    so the projection x @ waug directly yields [Wh | s_src | s_dst].
  - rid  = per-core global row indices [NS, 1] uint32.

Device (SPMD, per core):
  phase 1: build the FULL augmented table whs[4096, 520] bf16 in local DRAM
           via 512 bf16 matmuls (no transposes needed - xT comes in
           pre-transposed). Redundant across cores, but PE is otherwise idle
           and this removes the AllGather entirely.
  phase 2 (per 128-row adj tile): exact top-16 of each fp32 adj row (DVE
           max8/max_index/match_replace, jax tie-break), one indirect DMA
           gathering 17 table rows per node (16 neighbors + own row for
           s_src), leaky-relu scores + max-sub softmax (DVE+ACT), weighted
           sum as one strided DVE reduce over k, per-head 1/Z scale, ELU,
           bf16 store.

kernel(**inputs) takes FULL inputs and returns the FULL (4096, 512) fp32
output.
"""
import os
import sys

sys.path.insert(0, "/opt/trn_rl_repo")

import numpy as np

import concourse.bass as bass
import concourse.bacc as bacc
import concourse.mybir as mybir
from concourse.tile import TileContext
from concourse.bass_utils import run_bass_kernel_spmd
from concourse.masks import make_identity

NCORES = int(os.environ.get("KNL_NCORES", "8"))
N = 4096
DIN = 1024
DOUT = 512
H = 4
DH = 128
K = 16
NS = N // NCORES          # 512 rows per core
T = NS // 128             # 4 tiles of 128 rows per core
NT = N // 128             # 32 tiles in the full table
AUG = DOUT + 2 * H        # 520: [Wh | s_src | s_dst]
NEG_SLOPE = 0.2
FP = mybir.dt.float32
BF = mybir.dt.bfloat16
U32 = mybir.dt.uint32
BF_NP = mybir.dt.np(mybir.dt.bfloat16)


def _topk(nc, adj_t, m8a, m8b, idx):
    """Exact top-16 per row: two rounds of DVE max8/max_index, with
    match_replace eliminating round-1 winners (jax top_k tie-break)."""
    nc.vector.max(out=m8a[:], in_=adj_t[:])
    nc.vector.max_index(out=idx[:, 0:8], in_max=m8a[:], in_values=adj_t[:])
    nc.vector.match_replace(
        out=adj_t[:], in_to_replace=m8a[:], in_values=adj_t[:], imm_value=-1.0,
    )
    nc.vector.max(out=m8b[:], in_=adj_t[:])
    nc.vector.max_index(out=idx[:, 8:16], in_max=m8b[:], in_values=adj_t[:])


def build_program():
    nc = bacc.Bacc(
        "TRN2",
        target_bir_lowering=False,
        debug=False,
        num_devices=NCORES,
        num_swdge_queues=4,
    )

    adj_s = nc.dram_tensor("adj_s", [NS, N], FP, kind="ExternalInput")
    xT = nc.dram_tensor("xT", [DIN, N], BF, kind="ExternalInput")
    waug = nc.dram_tensor("waug", [DIN, AUG], BF, kind="ExternalInput")
    rid = nc.dram_tensor("rid", [NS, 1], U32, kind="ExternalInput")
    out_s = nc.dram_tensor("out_s", [NS, DOUT], BF, kind="ExternalOutput")

    whs = nc.dram_tensor("whs", [N, AUG], BF)  # local full table

    with TileContext(nc) as tc:
        with (
            tc.tile_pool(name="const", bufs=1) as cpool,
            tc.tile_pool(name="p1w", bufs=2) as p1w,
            tc.tile_pool(name="p1psA", bufs=2, space="PSUM") as p1psA,
            tc.tile_pool(name="p1psB", bufs=2, space="PSUM") as p1psB,
        ):
            ident = cpool.tile([128, 128], BF)
            make_identity(nc, ident[:])
            rid_sb = cpool.tile([128, T], U32)
            nc.sync.dma_start(rid_sb[:], rid.rearrange("(t p) o -> p (t o)", p=128))
            waug_sb = cpool.tile([128, 8, AUG], BF)
            nc.sync.dma_start(waug_sb[:], waug.rearrange("(c p) g -> p c g", p=128))

            # preload ALL adj tiles before the 8MB xT load: top-k only needs
            # adj, so the whole DVE top-k chain can run during the phase-1
            # table build instead of queueing its DMAs behind xT's.
            adj_tiles = []
            for t in range(T):
                at = cpool.tile([128, N], FP, tag=f"adj{t}")
                nc.sync.dma_start(at[:], adj_s[t * 128 : (t + 1) * 128, :])
                adj_tiles.append(at)
            # xT lives in its own pool scoped to phase 1: its 64KB/partition
            # is released before the phase-2 gather pool allocates, allowing
            # 4-deep gather double-buffering.
            xp_pool = tc.tile_pool(name="xp", bufs=1)
            xp = xp_pool.__enter__()
            # split per 128-row chunk so adj-tile loads can interleave on the
            # DMA queues instead of queueing behind one monolithic 8MB copy
            xT_sb = xp.tile([128, 8, N], BF)
            for c in range(8):
                nc.sync.dma_start(
                    xT_sb[:, c, :], xT[c * 128 : (c + 1) * 128, :]
                )

            # ---------------- phase 1: full augmented table ----------------
            for t in range(NT):
                psA = p1psA.tile([128, DOUT], FP, tag="psA")
                psB = p1psB.tile([128, 2 * H], FP, tag="psB")
                ncc = 1 if os.environ.get("KNL_SIM_THIN_P1") else 8
                for c in range(ncc):
                    lt = xT_sb[:, c, t * 128 : (t + 1) * 128]
                    nc.tensor.matmul(
                        out=psA[:], lhsT=lt, rhs=waug_sb[:, c, 0:DOUT],
                        start=(c == 0), stop=(c == ncc - 1),
                    )
                    nc.tensor.matmul(
                        out=psB[:], lhsT=lt, rhs=waug_sb[:, c, DOUT:AUG],
                        start=(c == 0), stop=(c == ncc - 1),
                    )
                wt = p1w.tile([128, AUG], BF, tag="wt")
                if os.environ.get("KNL_ACT_COPY", "1") == "1":
                    # Act engine is idle in phase 1; do the big PSUM->SBUF
                    # copy there so DVE stays free for phase-2 topk.
                    nc.scalar.activation(
                        out=wt[:, 0:DOUT], in_=psA[:],
                        func=mybir.ActivationFunctionType.Copy,
                    )
                else:
                    nc.vector.tensor_copy(wt[:, 0:DOUT], psA[:])
                nc.vector.tensor_copy(wt[:, DOUT:AUG], psB[:])
                nc.sync.dma_start(whs[t * 128 : (t + 1) * 128, :], wt[:])

            xp_pool.__exit__(None, None, None)  # free xT's 64KB/partition

            # ---------------- phase 2: per-tile topk/softmax/reduce ----------------
            with (
                tc.tile_pool(name="gp", bufs=4) as gp,
                tc.tile_pool(name="smallp", bufs=2) as smallp,
                tc.tile_pool(name="outp", bufs=2) as outp,
                tc.tile_pool(name="accp", bufs=2, space="PSUM") as accp,
            ):
                for t in range(T):
                    adj_t = adj_tiles[t]

                    m8a = smallp.tile([128, 8], FP, tag="m8a")
                    m8b = smallp.tile([128, 8], FP, tag="m8b")
                    idx = smallp.tile([128, K + 1], U32, tag="idx")
                    if os.environ.get("KNL_SIM_NO_TOPK"):  # sim what-if only
                        nc.vector.memset(idx[:], 0)
                        nc.vector.memset(m8a[:], 0.0)
                        nc.vector.memset(m8b[:], 0.0)
                    else:
                        _topk(nc, adj_t, m8a, m8b, idx)

                    # 17th gather row = own row (for s_src)
                    nc.gpsimd.tensor_copy(idx[:, K : K + 1], rid_sb[:, t : t + 1])


                    # one indirect DMA per gathered row; a single fused
                    # 17-offset indirect DMA passes CoreSim but crashes real
                    # HW (NRT_EXEC_UNIT_UNRECOVERABLE), so keep them split.
                    G = gp.tile([128, K + 1, AUG], BF, tag="G")
                    if os.environ.get("KNL_SIM_NO_GATHER"):  # sim what-if only
                        nc.vector.memset(G[:, 0, :], 0.5)
                    else:
                        # round-robin the 17 gathers over all 4 SWDGE queues;
                        # the sim ablation shows the serialized gather chain
                        # is the largest critical-path item (-96us if removed)
                        for k in range(K + 1):
                            bi = nc.gpsimd.indirect_dma_start(
                                out=G[:, k, :], out_offset=None, in_=whs[:],
                                in_offset=bass.IndirectOffsetOnAxis(
                                    ap=idx[:, k : k + 1], axis=0
                                ),
                            )
                            q = k % 4
                            if q:
                                bi.ins.queue = f"qPoolDynamic{q}"

                    # scores e[p,h,k] = leaky(s_src[own,h] + s_dst[idx[p,k],h])
                    S = smallp.tile([128, H, K], FP, tag="S")
                    nc.vector.tensor_tensor(
                        out=S[:],
                        in0=G[:, 0:K, DOUT + H : AUG].rearrange("p k h -> p h k"),
                        in1=G[:, K, DOUT : DOUT + H].to_broadcast([128, H, K]),
                        op=mybir.AluOpType.add,
                    )
                    E = smallp.tile([128, H, K], FP, tag="E")
                    nc.vector.scalar_tensor_tensor(
                        out=E[:], in0=S[:], scalar=NEG_SLOPE, in1=S[:],
                        op0=mybir.AluOpType.mult, op1=mybir.AluOpType.max,
                    )
                    M = smallp.tile([128, H], FP, tag="M")
                    nc.vector.tensor_reduce(
                        out=M[:], in_=E[:], axis=mybir.AxisListType.X,
                        op=mybir.AluOpType.max,
                    )
                    negM = smallp.tile([128, H], FP, tag="negM")
                    nc.vector.tensor_scalar(
                        out=negM[:], in0=M[:], scalar1=-1.0, scalar2=None,
                        op0=mybir.AluOpType.mult,
                    )
                    P = smallp.tile([128, H, K], BF, tag="P")
                    Z = smallp.tile([128, H], FP, tag="Z")
                    for h in range(H):
                        nc.scalar.activation(
                            out=P[:, h, :], in_=E[:, h, :],
                            func=mybir.ActivationFunctionType.Exp,
                            bias=negM[:, h : h + 1], scale=1.0,
                            accum_out=Z[:, h : h + 1],
                        )
                    rec = smallp.tile([128, H], FP, tag="rec")
                    nc.vector.reciprocal(out=rec[:], in_=Z[:])

                    # weighted sum: G[p,k,f] *= P[p,h(f),k] in place, then
                    # strided sum over k, then per-head 1/Z scale.
                    # on DVE, not gpsimd: Pool is the gather-trigger engine,
                    # and a 16us scale op there stalls the next tile's
                    # indirect-DMA triggers; in bf16 DVE does this in ~2us.
                    gview = G[:, 0:K, 0:DOUT].rearrange("p k (h c) -> p k h c", h=H)
                    nc.vector.tensor_tensor(
                        out=gview, in0=gview,
                        in1=P[:].rearrange("p h k -> p k h").to_broadcast(
                            [128, K, H, DH]
                        ),
                        op=mybir.AluOpType.mult,
                    )
                    # k-sum on PE (idle in phase 2): 16 PSUM-accumulated
                    # identity matmuls; frees DVE (the busiest engine) of the
                    # 8.7us/tile strided tensor_reduce.
                    acc = accp.tile([128, DOUT], FP, tag="acc")
                    for k in range(K):
                        nc.tensor.matmul(
                            out=acc[:], lhsT=ident[:], rhs=G[:, k, 0:DOUT],
                            start=(k == 0), stop=(k == K - 1),
                        )
                    osum = outp.tile([128, DOUT], FP, tag="osum")
                    nc.vector.tensor_tensor(
                        out=osum[:].rearrange("p (h c) -> p h c", h=H),
                        in0=acc[:].rearrange("p (h c) -> p h c", h=H),
                        in1=rec[:].to_broadcast([128, H, DH]),
                        op=mybir.AluOpType.mult,
                    )

                    # elu(x) = relu(x) + exp(min(x,0)) - 1
                    u = outp.tile([128, DOUT], FP, tag="u")
                    nc.vector.tensor_scalar(
                        out=u[:], in0=osum[:], scalar1=0.0, scalar2=None,
                        op0=mybir.AluOpType.min,
                    )
                    e1 = outp.tile([128, DOUT], FP, tag="e1")
                    nc.scalar.activation(
                        out=e1[:], in_=u[:], func=mybir.ActivationFunctionType.Exp,
                    )
                    r1 = outp.tile([128, DOUT], FP, tag="r1")
                    nc.scalar.activation(
                        out=r1[:], in_=osum[:], func=mybir.ActivationFunctionType.Relu,
                    )
                    o = outp.tile([128, DOUT], BF, tag="o")
                    nc.vector.scalar_tensor_tensor(
                        out=o[:], in0=e1[:], scalar=-1.0, in1=r1[:],
                        op0=mybir.AluOpType.add, op1=mybir.AluOpType.add,
                    )
                    nc.sync.dma_start(out_s[t * 128 : (t + 1) * 128, :], o[:])

    nc.compile()
    return nc


_NC_CACHE = None


def _get_program():
    global _NC_CACHE
    if _NC_CACHE is None:
        _NC_CACHE = build_program()
    return _NC_CACHE


def make_in_maps(x, adj, W, a):
    x = np.ascontiguousarray(np.asarray(x, dtype=np.float32))
    adj = np.ascontiguousarray(np.asarray(adj, dtype=np.float32))
    W = np.ascontiguousarray(np.asarray(W, dtype=np.float32))
    a = np.ascontiguousarray(np.asarray(a, dtype=np.float32))

    # weight packing (host): waug = [W.T | w_src | w_dst], xT = x.T (bf16)
    a_src, a_dst = a[0, :DH], a[0, DH:]
    W3 = W.reshape(H, DH, DIN)
    w_src = np.einsum("hkd,k->dh", W3, a_src)      # [DIN, H]
    w_dst = np.einsum("hkd,k->dh", W3, a_dst)      # [DIN, H]
    waug = np.concatenate([W.T, w_src, w_dst], axis=1).astype(BF_NP)
    xTb = np.ascontiguousarray(x.T).astype(BF_NP)  # [DIN, N]

    return [
        {
            "adj_s": adj[c * NS : (c + 1) * NS],
            "xT": xTb,
            "waug": waug,
            "rid": (np.arange(NS, dtype=np.uint32) + c * NS)[:, None],
        }
        for c in range(NCORES)
    ]


def kernel(x, adj, W, a, _trace=False):
    nc = _get_program()
    in_maps = make_in_maps(x, adj, W, a)
    res = run_bass_kernel_spmd(nc, in_maps, list(range(NCORES)), trace=_trace)
    out = np.concatenate(
        [res.results[c]["out_s"].astype(np.float32) for c in range(NCORES)], axis=0
    )
    if _trace:
        return out, res
    return out
